# revision 1
# baseline (speedup 1.0000x reference)
import numpy as np
import jax
import jax.numpy as jnp
from functools import partial
from jax.sharding import Mesh, PartitionSpec as P

try:
    from jax.experimental.shard_map import shard_map
except ImportError:
    from jax.shard_map import shard_map

# Problem constants (nn_GaussianMaskedMultiheadAttention): x [B,S,E], H heads.
B, S, E, H = 2, 4096, 512, 8
D = E // H
M = 8  # cores


def kernel(x, in_proj_w, in_proj_b, out_proj_w, out_proj_b, t):
    devs = jax.devices()[:M]
    mesh = Mesh(np.array(devs), ("m",))
    scale = 1.0 / float(np.sqrt(D))

    # Head-parallel layout: each core gets H/M heads of Q/K/V weights,
    # its slice of the Gaussian bias, and its column slice of out_proj.
    wq = np.asarray(in_proj_w[0:E]).reshape(H, D, E)
    wk = np.asarray(in_proj_w[E : 2 * E]).reshape(H, D, E)
    wv = np.asarray(in_proj_w[2 * E : 3 * E]).reshape(H, D, E)
    bq = np.asarray(in_proj_b[0:E]).reshape(H, D)
    bk = np.asarray(in_proj_b[E : 2 * E]).reshape(H, D)
    bv = np.asarray(in_proj_b[2 * E : 3 * E]).reshape(H, D)
    wo = np.asarray(out_proj_w).reshape(E, H, D).transpose(1, 0, 2)  # [H,E,D]
    s2 = (np.asarray(t, dtype=np.float32) ** 2) ** 2  # sigma^2 per head

    @jax.jit
    @partial(
        shard_map,
        mesh=mesh,
        in_specs=(
            P(), P("m"), P("m"), P("m"), P("m"), P("m"), P("m"), P("m"),
            P("m"), P(),
        ),
        out_specs=P(),
    )
    def f(x, wq, wk, wv, bq, bk, bv, wo, s2, ob):
        q = jnp.einsum("bse,hde->bhsd", x, wq) + bq[None, :, None, :]
        k = jnp.einsum("bse,hde->bhsd", x, wk) + bk[None, :, None, :]
        v = jnp.einsum("bse,hde->bhsd", x, wv) + bv[None, :, None, :]
        scores = jnp.einsum("bhqd,bhkd->bhqk", q, k) * scale
        idx = jnp.arange(S)
        dist2 = (idx[None, :] - idx[:, None]).astype(jnp.float32) ** 2
        bias = -dist2[None, None] / (2.0 * s2[None, :, None, None])
        attn = jax.nn.softmax(scores + bias, axis=-1)
        o = jnp.einsum("bhqk,bhkd->bhqd", attn, v)
        part = jnp.einsum("bhsd,hed->bse", o, wo)  # partial over local heads
        out = jax.lax.psum(part, "m")  # all-reduce after out_proj
        return out + ob[None, None, :]

    out = f(
        jnp.asarray(x, jnp.float32), jnp.asarray(wq), jnp.asarray(wk),
        jnp.asarray(wv), jnp.asarray(bq), jnp.asarray(bk), jnp.asarray(bv),
        jnp.asarray(wo), jnp.asarray(s2), jnp.asarray(out_proj_b, jnp.float32),
    )
    return np.asarray(jax.device_get(out), dtype=np.float32)


# revision 3
# speedup vs baseline: 1.1663x; 1.1663x over previous
import numpy as np
import jax
import jax.numpy as jnp
from functools import partial
from jax.sharding import Mesh, PartitionSpec as P

try:
    from jax.experimental.shard_map import shard_map
except ImportError:
    from jax.shard_map import shard_map

# Problem constants (nn_GaussianMaskedMultiheadAttention): x [B,S,E], H heads.
B, S, E, H = 2, 4096, 512, 8
D = E // H
M = 8  # cores


_F = None


def _build():
    global _F
    if _F is not None:
        return _F
    mesh = Mesh(np.array(jax.devices()[:M]), ("m",))
    scale = 1.0 / float(np.sqrt(D))

    @jax.jit
    @partial(
        shard_map,
        mesh=mesh,
        in_specs=(
            P(), P("m"), P("m"), P("m"), P("m"), P("m"), P("m"), P("m"),
            P("m"), P(),
        ),
        out_specs=P(),
    )
    def f(x, wq, wk, wv, bq, bk, bv, wo, s2, ob):
        q = jnp.einsum("bse,hde->bhsd", x, wq) + bq[None, :, None, :]
        k = jnp.einsum("bse,hde->bhsd", x, wk) + bk[None, :, None, :]
        v = jnp.einsum("bse,hde->bhsd", x, wv) + bv[None, :, None, :]
        scores = jnp.einsum("bhqd,bhkd->bhqk", q, k) * scale
        idx = jnp.arange(S)
        dist2 = (idx[None, :] - idx[:, None]).astype(jnp.float32) ** 2
        bias = -dist2[None, None] / (2.0 * s2[None, :, None, None])
        attn = jax.nn.softmax(scores + bias, axis=-1)
        o = jnp.einsum("bhqk,bhkd->bhqd", attn, v)
        part = jnp.einsum("bhsd,hed->bse", o, wo)  # partial over local heads
        out = jax.lax.psum(part, "m")  # all-reduce after out_proj
        return out + ob[None, None, :]

    _F = f
    return f


def kernel(x, in_proj_w, in_proj_b, out_proj_w, out_proj_b, t):
    f = _build()

    # Head-parallel layout: each core gets H/M heads of Q/K/V weights,
    # its slice of the Gaussian bias, and its column slice of out_proj.
    wq = np.asarray(in_proj_w[0:E]).reshape(H, D, E)
    wk = np.asarray(in_proj_w[E : 2 * E]).reshape(H, D, E)
    wv = np.asarray(in_proj_w[2 * E : 3 * E]).reshape(H, D, E)
    bq = np.asarray(in_proj_b[0:E]).reshape(H, D)
    bk = np.asarray(in_proj_b[E : 2 * E]).reshape(H, D)
    bv = np.asarray(in_proj_b[2 * E : 3 * E]).reshape(H, D)
    wo = np.asarray(out_proj_w).reshape(E, H, D).transpose(1, 0, 2)  # [H,E,D]
    s2 = (np.asarray(t, dtype=np.float32) ** 2) ** 2  # sigma^2 per head

    out = f(
        jnp.asarray(x, jnp.float32), jnp.asarray(wq), jnp.asarray(wk),
        jnp.asarray(wv), jnp.asarray(bq), jnp.asarray(bk), jnp.asarray(bv),
        jnp.asarray(wo), jnp.asarray(s2), jnp.asarray(out_proj_b, jnp.float32),
    )
    return np.asarray(jax.device_get(out), dtype=np.float32)
